# revision 1
# baseline (speedup 1.0000x reference)
"""Trainium kernel for nn_FNPGNNv5 (GAT-style message passing GNN).

Strategy: pure data parallelism — B=1024 independent samples sharded as
128 samples per NeuronCore across 8 cores; all weights (<1MB) replicated.
The per-sample graph is small (N=116 nodes, D=96), so each core runs the
full network on its batch shard and results are concatenated on host.

The model is expressed in JAX and compiled per-core via the neuron PJRT
backend (axon); inputs are placed per-device and the computation runs
SPMD on cores 0-7.
"""

import numpy as np
import jax
import jax.numpy as jnp
from functools import partial

# Hardcoded problem shapes (from the problem spec).
N, K, H, D, DH, IN_DIM, B = 116, 9, 4, 96, 24, 23, 1024
CD = K * D + D  # 960
NCORES = 8

NETWORK_MAP = {
    'DMN': [34, 35, 66, 67, 64, 65, 22, 23, 24, 25],
    'SMN': [0, 1, 56, 57, 68, 69],
    'VN': [42, 43, 44, 45, 46, 47, 48, 49, 50, 51, 52, 53],
    'SN': [28, 29, 30, 31, 32, 33],
    'FPN': [6, 7, 58, 59, 60, 61],
    'LN': [36, 37, 38, 39, 40, 41],
    'VAN': [10, 11, 14, 15],
    'BGN': [70, 71, 72, 73, 74, 75, 76, 77],
    'CereN': list(range(90, 116)),
}


def _bn_eval(x, g, b):
    return x * (g / jnp.sqrt(1.0 + 1e-5)) + b


def _layer_norm(x, g, b):
    mu = x.mean(-1, keepdims=True)
    var = ((x - mu) ** 2).mean(-1, keepdims=True)
    return (x - mu) / jnp.sqrt(var + 1e-5) * g + b


def _gat(h, adj, W, a, g, b):
    Bn, Nn, _ = h.shape
    Wh = (h @ W).reshape(Bn, Nn, H, DH)
    s = jnp.einsum('bnhd,hd->bnh', Wh, a[:, :DH])
    t = jnp.einsum('bnhd,hd->bnh', Wh, a[:, DH:])
    e = s[:, :, None, :] + t[:, None, :, :]  # [B, Ni, Nj, H]
    # exp(leaky_relu(e, 0.2)) == max(exp(e), exp(0.2 e)); multiplying by the
    # 0/1 adjacency mask afterwards is equivalent to the -inf mask + softmax
    # (every row has a self-loop, so no fully-masked rows).
    w = jnp.maximum(jnp.exp(e), jnp.exp(0.2 * e))
    w = w * (adj != 0.0)[..., None]
    attn = w / w.sum(axis=2, keepdims=True)
    out = jnp.einsum('bijh,bjhd->bihd', attn, Wh).reshape(Bn, Nn, D)
    return jax.nn.elu(_bn_eval(out, g, b)) + h @ W


def _forward(x, adj, W_enc, b_enc, g_in, be_in, vn_emb, W1, a1, g1, be1,
             W2, a2, g2, be2, W_vn, b_vn, pool, W_na1, b_na1, W_na2, b_na2,
             g_ln, be_ln, W_c1, b_c1, W_c2, b_c2, W_p1, b_p1, W_p2, b_p2):
    Bn = x.shape[0]
    h = _bn_eval(jax.nn.elu(x @ W_enc + b_enc), g_in, be_in)
    vn = vn_emb + h.mean(1, keepdims=True)
    h = _gat(h, adj, W1, a1, g1, be1)
    vn_new = jax.nn.elu(
        jnp.concatenate([vn, h.mean(1, keepdims=True)], -1) @ W_vn + b_vn)
    h = h + vn_new * 0.1
    h = _gat(h, adj, W2, a2, g2, be2) + h
    vn = jax.nn.elu(
        jnp.concatenate([vn_new, h.mean(1, keepdims=True)], -1) @ W_vn + b_vn)
    pooled = jnp.einsum('bnd,nk->bkd', h, pool)
    net_w = jax.nn.softmax(
        jnp.tanh(pooled @ W_na1 + b_na1) @ W_na2 + b_na2, axis=1)
    pooled = pooled * net_w
    flat = _layer_norm(
        jnp.concatenate([pooled.reshape(Bn, -1), vn[:, 0]], -1), g_ln, be_ln)
    logits = jax.nn.elu(flat @ W_c1 + b_c1) @ W_c2 + b_c2
    pr = jax.nn.elu(flat @ W_p1 + b_p1) @ W_p2 + b_p2
    proj = pr / jnp.maximum(jnp.linalg.norm(pr, axis=-1, keepdims=True), 1e-12)
    return logits, proj


_BATCH_KEYS = ('x', 'adj')
_compiled = None


def _get_compiled():
    global _compiled
    if _compiled is None:
        _compiled = jax.pmap(_forward, axis_name='cores')
    return _compiled


def kernel(**inputs):
    inputs = {k: np.asarray(v) for k, v in inputs.items()}
    devs = jax.devices()[:NCORES]

    # Shard batch across the 8 cores; replicate everything else.
    order = ['x', 'adj', 'W_enc', 'b_enc', 'g_in', 'be_in', 'vn_emb', 'W1',
             'a1', 'g1', 'be1', 'W2', 'a2', 'g2', 'be2', 'W_vn', 'b_vn',
             'pool', 'W_na1', 'b_na1', 'W_na2', 'b_na2', 'g_ln', 'be_ln',
             'W_c1', 'b_c1', 'W_c2', 'b_c2', 'W_p1', 'b_p1', 'W_p2', 'b_p2']
    args = []
    for name in order:
        v = inputs[name]
        if name in _BATCH_KEYS:
            args.append(v.reshape((NCORES, v.shape[0] // NCORES) + v.shape[1:]))
        else:
            args.append(np.broadcast_to(v, (NCORES,) + v.shape))
    fn = _get_compiled()
    logits, proj = fn(*args)
    logits = np.asarray(logits).reshape(B, 2)
    proj = np.asarray(proj).reshape(B, 64)
    return logits, proj


if __name__ == '__main__':
    rng = np.random.default_rng(0)
    ins = {
        'x': rng.standard_normal((B, N, IN_DIM), dtype=np.float32),
        'adj': rng.standard_normal((B, N, N), dtype=np.float32),
    }
    print('smoke test only; use test.py for the real check')


# revision 5
# speedup vs baseline: 1.8218x; 1.8218x over previous
"""Trainium kernel for nn_FNPGNNv5 (GAT-style message passing GNN).

Strategy: pure data parallelism — B=1024 independent samples sharded as
128 samples per NeuronCore across 8 cores; all weights (<1MB) replicated.
The per-sample graph is small (N=116 nodes, D=96), so each core runs the
full network on its batch shard and results are concatenated on host.

The model is expressed in JAX and compiled per-core via the neuron PJRT
backend (axon); inputs are placed per-device and the computation runs
SPMD on cores 0-7.
"""

import numpy as np
import jax
import jax.numpy as jnp
from functools import partial

# Hardcoded problem shapes (from the problem spec).
N, K, H, D, DH, IN_DIM, B = 116, 9, 4, 96, 24, 23, 1024
CD = K * D + D  # 960
NCORES = 8

NETWORK_MAP = {
    'DMN': [34, 35, 66, 67, 64, 65, 22, 23, 24, 25],
    'SMN': [0, 1, 56, 57, 68, 69],
    'VN': [42, 43, 44, 45, 46, 47, 48, 49, 50, 51, 52, 53],
    'SN': [28, 29, 30, 31, 32, 33],
    'FPN': [6, 7, 58, 59, 60, 61],
    'LN': [36, 37, 38, 39, 40, 41],
    'VAN': [10, 11, 14, 15],
    'BGN': [70, 71, 72, 73, 74, 75, 76, 77],
    'CereN': list(range(90, 116)),
}


def _bn_eval(x, g, b):
    return x * (g / jnp.sqrt(1.0 + 1e-5)) + b


def _layer_norm(x, g, b):
    mu = x.mean(-1, keepdims=True)
    var = ((x - mu) ** 2).mean(-1, keepdims=True)
    return (x - mu) / jnp.sqrt(var + 1e-5) * g + b


def _gat(h, mask, W, a, g, b):
    Bn, Nn, _ = h.shape
    Wh = (h @ W).reshape(Bn, Nn, H, DH)
    s = jnp.einsum('bnhd,hd->bnh', Wh, a[:, :DH])
    t = jnp.einsum('bnhd,hd->bnh', Wh, a[:, DH:])
    e = s[:, :, None, :] + t[:, None, :, :]  # [B, Ni, Nj, H]
    # Multiplying exp(lrelu(e)) by the 0/1 adjacency mask is equivalent to
    # the -inf mask + softmax (every row has a self-loop, so no fully-masked
    # rows).
    w = jnp.exp(jax.nn.leaky_relu(e, 0.2)) * mask[..., None]
    attn = w / w.sum(axis=2, keepdims=True)
    out = jnp.einsum('bijh,bjhd->bihd', attn, Wh).reshape(Bn, Nn, D)
    return jax.nn.elu(_bn_eval(out, g, b)) + h @ W


def _forward(x, adj_mask, W_enc, b_enc, g_in, be_in, vn_emb, W1, a1, g1, be1,
             W2, a2, g2, be2, W_vn, b_vn, pool, W_na1, b_na1, W_na2, b_na2,
             g_ln, be_ln, W_c1, b_c1, W_c2, b_c2, W_p1, b_p1, W_p2, b_p2):
    Bn = x.shape[0]
    mask = adj_mask.astype(jnp.float32)
    h = _bn_eval(jax.nn.elu(x @ W_enc + b_enc), g_in, be_in)
    vn = vn_emb + h.mean(1, keepdims=True)
    h = _gat(h, mask, W1, a1, g1, be1)
    vn_new = jax.nn.elu(
        jnp.concatenate([vn, h.mean(1, keepdims=True)], -1) @ W_vn + b_vn)
    h = h + vn_new * 0.1
    h = _gat(h, mask, W2, a2, g2, be2) + h
    vn = jax.nn.elu(
        jnp.concatenate([vn_new, h.mean(1, keepdims=True)], -1) @ W_vn + b_vn)
    pooled = jnp.einsum('bnd,nk->bkd', h, pool)
    net_w = jax.nn.softmax(
        jnp.tanh(pooled @ W_na1 + b_na1) @ W_na2 + b_na2, axis=1)
    pooled = pooled * net_w
    flat = _layer_norm(
        jnp.concatenate([pooled.reshape(Bn, -1), vn[:, 0]], -1), g_ln, be_ln)
    logits = jax.nn.elu(flat @ W_c1 + b_c1) @ W_c2 + b_c2
    pr = jax.nn.elu(flat @ W_p1 + b_p1) @ W_p2 + b_p2
    proj = pr / jnp.maximum(jnp.linalg.norm(pr, axis=-1, keepdims=True), 1e-12)
    return logits, proj


_BATCH_KEYS = ('x', 'adj')
_compiled = None


def _get_compiled():
    global _compiled
    if _compiled is None:
        _compiled = jax.pmap(_forward, axis_name='cores')
    return _compiled


def kernel(**inputs):
    inputs = {k: np.asarray(v) for k, v in inputs.items()}

    # The adjacency values are never used by the model — only (adj != 0).
    # Ship a uint8 mask (4x less tunnel traffic, bit-exact result).
    adj_mask = (inputs['adj'] != 0.0).astype(np.uint8)

    # Shard batch across the 8 cores; replicate everything else.
    order = ['x', 'adj', 'W_enc', 'b_enc', 'g_in', 'be_in', 'vn_emb', 'W1',
             'a1', 'g1', 'be1', 'W2', 'a2', 'g2', 'be2', 'W_vn', 'b_vn',
             'pool', 'W_na1', 'b_na1', 'W_na2', 'b_na2', 'g_ln', 'be_ln',
             'W_c1', 'b_c1', 'W_c2', 'b_c2', 'W_p1', 'b_p1', 'W_p2', 'b_p2']
    args = []
    for name in order:
        v = adj_mask if name == 'adj' else inputs[name]
        if name in _BATCH_KEYS:
            args.append(v.reshape((NCORES, v.shape[0] // NCORES) + v.shape[1:]))
        else:
            args.append(np.broadcast_to(v, (NCORES,) + v.shape))
    fn = _get_compiled()
    logits, proj = fn(*args)
    logits = np.asarray(logits).reshape(B, 2)
    proj = np.asarray(proj).reshape(B, 64)
    return logits, proj


if __name__ == '__main__':
    rng = np.random.default_rng(0)
    ins = {
        'x': rng.standard_normal((B, N, IN_DIM), dtype=np.float32),
        'adj': rng.standard_normal((B, N, N), dtype=np.float32),
    }
    print('smoke test only; use test.py for the real check')


# revision 6
# speedup vs baseline: 14.9942x; 8.2302x over previous
"""Trainium kernel for nn_FNPGNNv5 (GAT-style message passing GNN).

Strategy: pure data parallelism — B=1024 independent samples sharded as
128 samples per NeuronCore across 8 cores; all weights (<1MB) replicated.
The per-sample graph is small (N=116 nodes, D=96), so each core runs the
full network on its batch shard and results are concatenated on host.

Transfer optimizations (the axon tunnel is latency/bandwidth limited):
 - the adjacency VALUES are never used by the model, only (adj != 0), so
   the 55MB fp32 adj is shipped as a 1.7MB bit-packed mask and unpacked
   on device (bit-exact);
 - all 30 weight arrays are concatenated into one flat fp32 vector and
   sliced back out at trace time (1 transfer instead of 30 per device).
"""

import numpy as np
import jax
import jax.numpy as jnp

# Hardcoded problem shapes (from the problem spec).
N, K, H, D, DH, IN_DIM, B = 116, 9, 4, 96, 24, 23, 1024
CD = K * D + D  # 960
NCORES = 8
NPACK = (N + 7) // 8  # 15 packed bytes per adjacency row

# (name, shape) for every non-batch input, in flat-pack order.
_W_SPECS = [
    ('W_enc', (IN_DIM, D)), ('b_enc', (D,)), ('g_in', (D,)), ('be_in', (D,)),
    ('vn_emb', (1, 1, D)),
    ('W1', (D, D)), ('a1', (H, 2 * DH)), ('g1', (D,)), ('be1', (D,)),
    ('W2', (D, D)), ('a2', (H, 2 * DH)), ('g2', (D,)), ('be2', (D,)),
    ('W_vn', (2 * D, D)), ('b_vn', (D,)),
    ('pool', (N, K)),
    ('W_na1', (D, 32)), ('b_na1', (32,)), ('W_na2', (32, 1)), ('b_na2', (1,)),
    ('g_ln', (CD,)), ('be_ln', (CD,)),
    ('W_c1', (CD, 192)), ('b_c1', (192,)), ('W_c2', (192, 2)), ('b_c2', (2,)),
    ('W_p1', (CD, 128)), ('b_p1', (128,)), ('W_p2', (128, 64)), ('b_p2', (64,)),
]


def _bn_eval(x, g, b):
    return x * (g / jnp.sqrt(1.0 + 1e-5)) + b


def _layer_norm(x, g, b):
    mu = x.mean(-1, keepdims=True)
    var = ((x - mu) ** 2).mean(-1, keepdims=True)
    return (x - mu) / jnp.sqrt(var + 1e-5) * g + b


def _gat(h, mask, W, a, g, b):
    Bn, Nn, _ = h.shape
    Wh = (h @ W).reshape(Bn, Nn, H, DH)
    s = jnp.einsum('bnhd,hd->bnh', Wh, a[:, :DH])
    t = jnp.einsum('bnhd,hd->bnh', Wh, a[:, DH:])
    e = s[:, :, None, :] + t[:, None, :, :]  # [B, Ni, Nj, H]
    # Multiplying exp(lrelu(e)) by the 0/1 adjacency mask is equivalent to
    # the -inf mask + softmax (every row has a self-loop, so no fully-masked
    # rows).
    w = jnp.exp(jax.nn.leaky_relu(e, 0.2)) * mask[..., None]
    attn = w / w.sum(axis=2, keepdims=True)
    out = jnp.einsum('bijh,bjhd->bihd', attn, Wh).reshape(Bn, Nn, D)
    return jax.nn.elu(_bn_eval(out, g, b)) + h @ W


def _forward(x, mask_packed, wflat):
    # Unpack weights (static slices — free at run time).
    ws = {}
    off = 0
    for name, shape in _W_SPECS:
        sz = int(np.prod(shape))
        ws[name] = wflat[off:off + sz].reshape(shape)
        off += sz

    # Unpack the bit-packed adjacency mask: [Bn, N, NPACK] uint8 -> [Bn, N, N].
    shifts = jnp.arange(7, -1, -1, dtype=jnp.int32)  # MSB-first (np.packbits)
    bits = (mask_packed.astype(jnp.int32)[..., None] >> shifts) & 1
    mask = bits.reshape(x.shape[0], N, NPACK * 8)[:, :, :N].astype(jnp.float32)

    Bn = x.shape[0]
    h = _bn_eval(jax.nn.elu(x @ ws['W_enc'] + ws['b_enc']),
                 ws['g_in'], ws['be_in'])
    vn = ws['vn_emb'] + h.mean(1, keepdims=True)
    h = _gat(h, mask, ws['W1'], ws['a1'], ws['g1'], ws['be1'])
    vn_new = jax.nn.elu(
        jnp.concatenate([vn, h.mean(1, keepdims=True)], -1) @ ws['W_vn']
        + ws['b_vn'])
    h = h + vn_new * 0.1
    h = _gat(h, mask, ws['W2'], ws['a2'], ws['g2'], ws['be2']) + h
    vn = jax.nn.elu(
        jnp.concatenate([vn_new, h.mean(1, keepdims=True)], -1) @ ws['W_vn']
        + ws['b_vn'])
    pooled = jnp.einsum('bnd,nk->bkd', h, ws['pool'])
    net_w = jax.nn.softmax(
        jnp.tanh(pooled @ ws['W_na1'] + ws['b_na1']) @ ws['W_na2']
        + ws['b_na2'], axis=1)
    pooled = pooled * net_w
    flat = _layer_norm(
        jnp.concatenate([pooled.reshape(Bn, -1), vn[:, 0]], -1),
        ws['g_ln'], ws['be_ln'])
    logits = jax.nn.elu(flat @ ws['W_c1'] + ws['b_c1']) @ ws['W_c2'] + ws['b_c2']
    pr = jax.nn.elu(flat @ ws['W_p1'] + ws['b_p1']) @ ws['W_p2'] + ws['b_p2']
    proj = pr / jnp.maximum(jnp.linalg.norm(pr, axis=-1, keepdims=True), 1e-12)
    return logits, proj


_compiled = None


def _get_compiled():
    global _compiled
    if _compiled is None:
        _compiled = jax.pmap(_forward, axis_name='cores')
    return _compiled


def pack_inputs(inputs):
    """Host-side prep: shard x, bit-pack adj mask, flatten weights."""
    x = np.ascontiguousarray(inputs['x'], dtype=np.float32)
    mask = (np.asarray(inputs['adj']) != 0.0)
    mask_packed = np.packbits(mask, axis=-1)  # [B, N, NPACK] uint8
    wflat = np.concatenate(
        [np.asarray(inputs[name], dtype=np.float32).reshape(-1)
         for name, _ in _W_SPECS])
    xs = x.reshape(NCORES, B // NCORES, N, IN_DIM)
    ms = mask_packed.reshape(NCORES, B // NCORES, N, NPACK)
    wf = np.broadcast_to(wflat, (NCORES,) + wflat.shape)
    return xs, ms, wf


def kernel(**inputs):
    xs, ms, wf = pack_inputs(inputs)
    fn = _get_compiled()
    logits, proj = fn(xs, ms, wf)
    return (np.asarray(logits).reshape(B, 2),
            np.asarray(proj).reshape(B, 64))
